# revision 8
# baseline (speedup 1.0000x reference)
"""Trainium2 Bass kernel for the Powderworld BehaviorFluidFlow step (v3).

Contract: kernel(**inputs) takes the FULL unsharded inputs
  world         (16, 20, 512, 512) f32
  rand_movement (16, 1, 512, 512) f32
  rand_interact (16, 1, 512, 512) f32   (unused by the reference)
  rand_element  (16, 1, 512, 512) f32   (unused by the reference)
and returns the FULL (16, 20, 512, 512) f32 output.

Sharding: data-parallel over batch; core k processes batches [2k, 2k+1].
All roll-based neighbor access is along W (axis 3), which stays local.

Layout per (batch, 128-row h-tile): channels split into group a = 6 rows
{0:id, 1:density, 2:gravity, 6:momentum, 8:did-gravity, E} and group b =
the 15 payload channels {3,4,5,7,9..19}; each group is one SBUF tile
(128, nch, 514) with one halo column per side for the circular W wrap.
E = "id in {empty,water,gas,lava,acid,agents}" is computed ONCE per tile
from the input ids (f32 exponent trick: (id+127)<<23 bitcast to f32 is
2^id; back to int gives 1<<id; AND 54025, nonzero) and then BLENDED like
any other channel, so pass 2 and the momentum fixup read it for free
(membership commutes with the blend, which only moves values).

Each pass computes single-channel move masks (a-mask = "pixel takes the
in-direction neighbor", b-mask = its shift; provably disjoint), then
blends with an Act-engine plain copy plus two DVE predicated copies, the
f32 mask broadcast across channels via a step-0 access pattern.  The mask
and-chains and 2+2 payload channels' arithmetic blends per pass run on the
GPSIMD engine -- on real HW GPSIMD runs at ~2.7x the cost model (vs DVE's
~1.94x), so the balance point leaves DVE with more work than the model
alone would suggest.
Store DMAs issue from the SP queue so the Act sequencer never blocks on
descriptor generation.  Stages are software-pipelined by emission order so
DVE always has blend work while GPSIMD finishes a mask chain.
"""
import sys

if '/opt/trn_rl_repo' not in sys.path:
    sys.path.insert(0, '/opt/trn_rl_repo')

import numpy as np
import concourse.bacc as bacc
import concourse.mybir as mybir
import concourse.tile as tile
from concourse.bass_utils import run_bass_kernel_spmd

A = mybir.AluOpType
F32 = mybir.dt.float32

B, C, H, W = 16, 20, 512, 512
N_CORES = 8
BPC = B // N_CORES
P = 128

_nc_cache = {}

# engine-assignment config (tuned via TimelineSim sweep)
CFG = {
    'npc1': 2,        # payload channels on GPSIMD, pass 1
    'npc2': 2,        # payload channels on GPSIMD, pass 2
    'fsc_pool': False,  # FSC fold (FS cmp 0.5 & DN) on Pool vs DVE
    'en_pool': False,   # EN = E*NDG on Pool vs DVE
    'memb_pool': False,  # membership chain on Pool vs DVE
    'store_act': False,  # store DMAs on Act queue vs SP queue
    'dn_pool': False,   # DN density compare on Pool vs DVE
    'tail_dve': False,  # last-iteration pass-2 blend fully on DVE
}


def build_kernel(bpc=BPC, c=C, h=H, w=W):
    key = (bpc, c, h, w, tuple(sorted(CFG.items())))
    if key in _nc_cache:
        return _nc_cache[key]

    nc = bacc.Bacc("TRN2", target_bir_lowering=False, debug=False,
                   num_devices=N_CORES)
    world = nc.dram_tensor("world", [bpc, c, h, w], F32, kind="ExternalInput")
    rand = nc.dram_tensor("rand", [bpc, h, w], F32, kind="ExternalInput")
    out = nc.dram_tensor("out", [bpc, c, h, w], F32, kind="ExternalOutput")

    WH = w + 2          # haloed width; data in cols [1, w], halos at 0 and w+1
    n_ht = h // P
    MAIN = slice(1, w + 1)
    ca, cb = 6, c - 5   # a = 5 mask channels + E, b = 15 payload channels
    IE = 5              # E row within group a
    NPC = {1: CFG['npc1'], 2: CFG['npc2']}

    def cbd(which, i):
        # final iteration: everything on DVE so the GPSIMD tail never
        # outlives the predicated-copy stream
        if CFG.get('tail_dve') and which == 2 and i == bpc * (h // P) - 1:
            return cb
        return cb - NPC[which]

    def store_eng():
        return nc.scalar if CFG['store_act'] else nc.sync

    # membership set {empty, water, lava, gas, acid, agentK, agentL}
    # = ids {0, 3, 8, 9, 12, 14, 15} = bits of 54025
    MBITS = 54025

    iters = [(b, t) for b in range(bpc) for t in range(n_ht)]
    n = len(iters)
    st = [dict() for _ in range(n)]   # per-iteration tile refs

    with tile.TileContext(nc) as tc:
        with tc.tile_pool(name="ga", bufs=4) as gap, \
             tc.tile_pool(name="gb", bufs=2) as gbp, \
             tc.tile_pool(name="out2", bufs=1) as o2p, \
             tc.tile_pool(name="mk", bufs=10) as mk, \
             tc.tile_pool(name="it", bufs=3) as itp, \
             tc.tile_pool(name="dbl", bufs=2) as dblp, \
             tc.tile_pool(name="amf", bufs=5) as amfp, \
             tc.tile_pool(name="ami", bufs=5) as amip, \
             tc.tile_pool(name="pb", bufs=4) as pbp, \
             tc.tile_pool(name="rp", bufs=3) as rp:

            def membership(ch0, out_tile):
                """out_tile = 1.0 where id in bits(MBITS) else 0.0."""
                eng = nc.gpsimd if CFG['memb_pool'] else nc.vector
                IT = itp.tile([P, w], mybir.dt.int32, tag="it")
                VT = itp.tile([P, w], mybir.dt.int32, tag="it")
                eng.tensor_copy(IT[:], ch0)
                eng.tensor_scalar(IT[:], IT[:], 8388608, 1065353216,
                                  A.mult, A.add)
                eng.tensor_copy(VT[:], IT[:].bitcast(F32))
                eng.tensor_scalar(VT[:], VT[:], MBITS, None, A.bitwise_and)
                eng.tensor_scalar(out_tile[:], VT[:], 0, None, A.is_gt)

            def loads_a(i):
                b, t = iters[i]
                hs = slice(t * P, (t + 1) * P)
                s = st[i]
                s['INa'] = gap.tile([P, ca, WH], F32, tag="ga", name=f"INa{i}")
                s['RAND'] = rp.tile([P, w], F32, tag="rand", name=f"RAND{i}")
                T = s['INa']
                nc.sync.dma_start(T[:, 0:3, MAIN],
                                  world[b, 0:3, hs, :].rearrange("c p w -> p c w"))
                nc.sync.dma_start(T[:, 3:4, MAIN],
                                  world[b, 6:7, hs, :].rearrange("c p w -> p c w"))
                nc.sync.dma_start(T[:, 4:5, MAIN],
                                  world[b, 8:9, hs, :].rearrange("c p w -> p c w"))
                nc.sync.dma_start(s['RAND'][:], rand[b, hs, :])

            def prep_a(i):
                """E membership + group-a halo columns (emit after the DMAs
                have had time to land so the DVE queue never stalls on them)."""
                T = st[i]['INa']
                membership(T[:, 0, MAIN], T[:, IE, MAIN])
                nc.scalar.copy(T[:, :, 0:1], T[:, :, w:w + 1])
                nc.scalar.copy(T[:, :, w + 1:w + 2], T[:, :, 1:2])

            def loads_b(i):
                b, t = iters[i]
                hs = slice(t * P, (t + 1) * P)
                s = st[i]
                s['INb'] = gbp.tile([P, cb, WH], F32, tag="gb", name=f"INb{i}")
                T = s['INb']
                nc.sync.dma_start(T[:, 0:3, MAIN],
                                  world[b, 3:6, hs, :].rearrange("c p w -> p c w"))
                nc.sync.dma_start(T[:, 3:4, MAIN],
                                  world[b, 7:8, hs, :].rearrange("c p w -> p c w"))
                nc.sync.dma_start(T[:, 4:cb, MAIN],
                                  world[b, 9:c, hs, :].rearrange("c p w -> p c w"))
                nc.scalar.copy(T[:, :, 0:1], T[:, :, w:w + 1])
                nc.scalar.copy(T[:, :, w + 1:w + 2], T[:, :, 1:2])

            def mask_head(i, which):
                """Move-mask chain up to DBL (mostly GPSIMD); group-a rows:
                0=id, 1=density, 2=gravity, 3=momentum(ch6), 4=didg(ch8), 5=E.

                which=1: nbr = j-1 (cur at 0:w), overlap-shift = j+1.
                which=2: nbr = j+1 (cur at 2:w+2), overlap-shift = j-1.
                """
                s = st[i]
                cur = s['INa'] if which == 1 else s['O1a']
                nbr = slice(0, w) if which == 1 else slice(2, w + 2)
                RAND = s['RAND']
                FS = mk.tile([P, w], F32, tag="mk", name=f"FS{which}_{i}")
                AIR = mk.tile([P, w], F32, tag="mk", name=f"AIR{which}_{i}")
                EN = mk.tile([P, w], F32, tag="mk", name=f"EN{which}_{i}")
                NDG = mk.tile([P, w], F32, tag="mk", name=f"NDG{which}_{i}")
                GB = mk.tile([P, w], F32, tag="mk", name=f"GB{which}_{i}")
                DN = mk.tile([P, w], F32, tag="mk", name=f"DN{which}_{i}")
                DBL = dblp.tile([P, WH], F32, tag="dbl", name=f"DBL{which}_{i}")
                s[f'DBL{which}'] = DBL

                if which == 1:
                    nc.gpsimd.tensor_tensor(FS[:], RAND[:], cur[:, 3, MAIN],
                                            A.add)
                else:
                    # DVE add so the pass-2 chain start never waits on the
                    # (possibly still draining) GPSIMD queue
                    nc.vector.tensor_tensor(FS[:], RAND[:], cur[:, 3, MAIN],
                                            A.add)
                    # + nfm = 2*b1 after pass 1
                    nc.vector.scalar_tensor_tensor(FS[:], s['A1'][:, 2:w + 2],
                                                   2.0, FS[:], A.mult, A.add)
                nc.vector.tensor_scalar(AIR[:], cur[:, 0, MAIN], 13.5, None,
                                        A.is_gt)
                nc.vector.scalar_tensor_tensor(NDG[:], cur[:, 4, MAIN], 0.5,
                                               AIR[:], A.is_lt, A.logical_or)
                dn_eng = nc.gpsimd if CFG['dn_pool'] else nc.vector
                dn_eng.tensor_tensor(DN[:], cur[:, 1, MAIN], cur[:, 1, nbr],
                                     A.is_gt)
                # gravity is exactly 0/1 so mult == and for the pair test
                nc.gpsimd.tensor_tensor(GB[:], cur[:, 2, MAIN], cur[:, 2, nbr],
                                        A.mult)
                en_eng = nc.gpsimd if CFG['en_pool'] else nc.vector
                en_eng.tensor_tensor(EN[:], cur[:, IE, MAIN], NDG[:], A.mult)
                cmp_op = A.is_gt if which == 1 else A.is_le
                fsc_eng = nc.gpsimd if CFG['fsc_pool'] else nc.vector
                fsc_eng.scalar_tensor_tensor(FS[:], FS[:], 0.5, DN[:],
                                             cmp_op, A.logical_and)
                nc.gpsimd.tensor_tensor(FS[:], FS[:], EN[:], A.mult)
                nc.gpsimd.tensor_tensor(DBL[:, MAIN], FS[:], GB[:], A.mult)
                if which == 1:
                    nc.gpsimd.tensor_copy(DBL[:, w + 1:w + 2], DBL[:, 1:2])
                else:
                    nc.gpsimd.tensor_copy(DBL[:, 0:1], DBL[:, w:w + 1])

            def mask_tail(i, which):
                """Overlap removal + halo columns for the real move mask."""
                s = st[i]
                DBL = s[f'DBL{which}']
                AMf = amfp.tile([P, WH], F32, tag="amf", name=f"AMf{which}_{i}")
                if which == 1:
                    nc.vector.scalar_tensor_tensor(AMf[:, MAIN], DBL[:, 2:w + 2],
                                                   0.0, DBL[:, MAIN],
                                                   A.is_equal, A.logical_and)
                else:
                    nc.vector.scalar_tensor_tensor(AMf[:, MAIN], DBL[:, 0:w],
                                                   0.0, DBL[:, MAIN],
                                                   A.is_equal, A.logical_and)
                nc.vector.tensor_copy(AMf[:, 0:1], AMf[:, w:w + 1])
                nc.vector.tensor_copy(AMf[:, w + 1:w + 2], AMf[:, 1:2])
                # the HW BIR verifier requires integer-typed cp masks
                AM = amip.tile([P, WH], mybir.dt.int8, tag="ami",
                               name=f"AM{which}_{i}")
                nc.vector.tensor_copy(AM[:], AMf[:])
                s[f'A{which}'] = AMf
                s[f'AM{which}'] = AM

            def pool_blend(s, which, cur, curch, outv):
                """Exact one-channel blend on GPSIMD:
                out = cur*(1-a-b) + a*nbr + b*opp (masks exactly 0/1)."""
                AMf = s[f'A{which}']
                nbr = slice(0, w) if which == 1 else slice(2, w + 2)
                opp = slice(2, w + 2) if which == 1 else slice(0, w)
                NM = s[f'NM{which}']
                X = pbp.tile([P, w], F32, tag="pb", name=f"X{which}")
                nc.gpsimd.tensor_tensor(X[:], cur[:, curch, MAIN], NM[:], A.mult)
                nc.gpsimd.tensor_tensor(outv, cur[:, curch, nbr], AMf[:, MAIN],
                                        A.mult)
                nc.gpsimd.tensor_tensor(outv, outv, X[:], A.add)
                nc.gpsimd.tensor_tensor(X[:], cur[:, curch, opp], AMf[:, opp],
                                        A.mult)
                nc.gpsimd.tensor_tensor(outv, outv, X[:], A.add)

            def make_nm(s, which):
                # NM = 1 - a - b (exactly 0 where the pixel moves, else 1)
                AMf = s[f'A{which}']
                opp = slice(2, w + 2) if which == 1 else slice(0, w)
                NM = pbp.tile([P, w], F32, tag="pb", name=f"NM{which}")
                nc.gpsimd.tensor_tensor(NM[:], AMf[:, MAIN], AMf[:, opp], A.add)
                nc.gpsimd.tensor_scalar(NM[:], NM[:], -1.0, 1.0, A.mult, A.add)
                s[f'NM{which}'] = NM

            def blend1_dve(i):
                s = st[i]
                A1 = s['AM1']
                s['O1a'] = gap.tile([P, ca, WH], F32, tag="ga", name=f"O1a{i}")
                s['O1b'] = gbp.tile([P, cb, WH], F32, tag="gb", name=f"O1b{i}")
                for IN, O1, nch in ((s['INa'], s['O1a'], ca),
                                    (s['INb'], s['O1b'], cbd(1, i))):
                    am = A1[:, MAIN].unsqueeze(1).broadcast_to((P, nch, w))
                    bm = A1[:, 2:w + 2].unsqueeze(1).broadcast_to((P, nch, w))
                    nc.scalar.copy(O1[:, 0:nch, MAIN], IN[:, 0:nch, MAIN])
                    nc.vector.copy_predicated(O1[:, 0:nch, MAIN], am,
                                              IN[:, 0:nch, 0:w])
                    nc.vector.copy_predicated(O1[:, 0:nch, MAIN], bm,
                                              IN[:, 0:nch, 2:w + 2])
                nc.scalar.copy(s['O1a'][:, :, 0:1], s['O1a'][:, :, w:w + 1])
                nc.scalar.copy(s['O1a'][:, :, w + 1:w + 2], s['O1a'][:, :, 1:2])
                O1b = s['O1b']
                d1 = cbd(1, i)
                nc.scalar.copy(O1b[:, 0:d1, 0:1], O1b[:, 0:d1, w:w + 1])
                nc.scalar.copy(O1b[:, 0:d1, w + 1:w + 2], O1b[:, 0:d1, 1:2])

            def blend1_pool(i):
                s = st[i]
                d1 = cbd(1, i)
                if d1 >= cb:
                    return
                make_nm(s, 1)
                for k in range(d1, cb):
                    pool_blend(s, 1, s['INb'], k, s['O1b'][:, k, MAIN])
                O1b = s['O1b']
                nc.scalar.copy(O1b[:, d1:cb, 0:1], O1b[:, d1:cb, w:w + 1])
                nc.scalar.copy(O1b[:, d1:cb, w + 1:w + 2],
                               O1b[:, d1:cb, 1:2])

            def blend2(i):
                b, t = iters[i]
                hs = slice(t * P, (t + 1) * P)
                s = st[i]
                A2 = s['AM2']
                s['O2'] = o2p.tile([P, ca + cb, WH], F32, tag="out2",
                                   name=f"O2_{i}")
                O2 = s['O2']
                d2 = cbd(2, i)
                groups = [(s['O1a'], 0, slice(0, ca), ca)]
                if i == n - 1:
                    # final iteration: split the payload cp group so its store
                    # transfers overlap the second half's predicated copies
                    h1 = 7
                    groups += [(s['O1b'], 0, slice(ca, ca + h1), h1),
                               (s['O1b'], h1, slice(ca + h1, ca + d2), d2 - h1)]
                else:
                    groups += [(s['O1b'], 0, slice(ca, ca + d2), d2)]
                for O1, r0, o2sl, nch in groups:
                    am = A2[:, MAIN].unsqueeze(1).broadcast_to((P, nch, w))
                    bm = A2[:, 0:w].unsqueeze(1).broadcast_to((P, nch, w))
                    nc.scalar.copy(O2[:, o2sl, MAIN], O1[:, r0:r0 + nch, MAIN])
                    nc.vector.copy_predicated(O2[:, o2sl, MAIN], am,
                                              O1[:, r0:r0 + nch, 2:w + 2])
                    nc.vector.copy_predicated(O2[:, o2sl, MAIN], bm,
                                              O1[:, r0:r0 + nch, 0:w])
                    if i == n - 1 and r0 == 0 and O1 is s['O1b']:
                        # ship ch9-11 while rows 7..d2 are still copying
                        store_eng().dma_start(
                            out[b, 9:12, hs, :].rearrange("c p w -> p c w"),
                            O2[:, ca + 4:ca + 7, MAIN])
                # store all channels except 6 (row 3; fixed up in fx) and E.
                # O2 row order is [0,1,2,6,8,E | 3,4,5,7 | 9..19].
                store_eng().dma_start(out[b, 0:3, hs, :].rearrange("c p w -> p c w"),
                                  O2[:, 0:3, MAIN])
                store_eng().dma_start(out[b, 8:9, hs, :].rearrange("c p w -> p c w"),
                                  O2[:, 4:5, MAIN])
                store_eng().dma_start(out[b, 3:6, hs, :].rearrange("c p w -> p c w"),
                                  O2[:, ca:ca + 3, MAIN])
                store_eng().dma_start(out[b, 7:8, hs, :].rearrange("c p w -> p c w"),
                                  O2[:, ca + 3:ca + 4, MAIN])
                if i == n - 1:
                    # final iteration: remaining DVE-blended payload rows
                    # (ch9-11 already shipped mid-blend)
                    d2l = cbd(2, i)
                    store_eng().dma_start(
                        out[b, 12:9 + (d2l - 4), hs, :].rearrange("c p w -> p c w"),
                        O2[:, ca + 7:ca + d2l, MAIN])

            def blend2_pool(i):
                b, t = iters[i]
                hs = slice(t * P, (t + 1) * P)
                s = st[i]
                O2 = s['O2']
                d2 = cbd(2, i)
                if d2 < cb:
                    make_nm(s, 2)
                    for k in range(d2, cb):
                        pool_blend(s, 2, s['O1b'], k, O2[:, ca + k, MAIN])
                d2l = cbd(2, i)
                lo = d2l if i == n - 1 else 4
                if lo < cb:
                    store_eng().dma_start(
                        out[b, 9 + (lo - 4):c, hs, :].rearrange("c p w -> p c w"),
                        O2[:, ca + lo:ca + cb, MAIN])

            def fixup(i):
                b, t = iters[i]
                hs = slice(t * P, (t + 1) * P)
                s = st[i]
                O2 = s['O2']
                NF = mk.tile([P, w], F32, tag="mk")
                FLI = amip.tile([P, w], mybir.dt.int8, tag="ami", name=f"FLI{i}")
                # nf = 2*b1 - 2*b2 (f32 masks, exact small integers)
                nc.gpsimd.tensor_tensor(NF[:], s['A1'][:, 2:w + 2],
                                        s['A2'][:, 0:w], A.subtract)
                nc.gpsimd.tensor_scalar(NF[:], NF[:], 2.0, None, A.mult)
                nc.scalar.copy(FLI[:], O2[:, IE, MAIN])
                nc.vector.copy_predicated(O2[:, 3, MAIN], FLI[:], NF[:])
                store_eng().dma_start(out[b, 6, hs, :], O2[:, 3, MAIN])

            # ---- software-pipelined emission -------------------------------
            loads_a(0)
            loads_b(0)
            if n > 1:
                loads_a(1)
            prep_a(0)
            mask_head(0, 1)
            mask_tail(0, 1)
            blend1_dve(0)
            if n > 1:
                loads_b(1)
                prep_a(1)
            for i in range(n):
                if i + 2 < n:
                    loads_a(i + 2)
                mask_head(i, 2)
                if i + 1 < n:
                    mask_head(i + 1, 1)
                mask_tail(i, 2)
                blend1_pool(i)
                blend2(i)
                blend2_pool(i)
                if i + 2 < n:
                    prep_a(i + 2)
                if i + 1 < n:
                    mask_tail(i + 1, 1)
                    blend1_dve(i + 1)
                if i + 2 < n:
                    loads_b(i + 2)
                fixup(i)

    nc.compile()
    _nc_cache[key] = nc
    return nc


def kernel(world, rand_movement, rand_interact, rand_element):
    del rand_interact, rand_element
    nc = build_kernel()
    in_maps = []
    for k in range(N_CORES):
        bs = slice(k * BPC, (k + 1) * BPC)
        in_maps.append({
            "world": np.ascontiguousarray(world[bs]),
            "rand": np.ascontiguousarray(rand_movement[bs, 0]),
        })
    res = run_bass_kernel_spmd(nc, in_maps, list(range(N_CORES)))
    return np.concatenate([res.results[k]["out"] for k in range(N_CORES)], axis=0)
